# revision 1
# baseline (speedup 1.0000x reference)
"""Trainium2 Bass kernel: masked-LSTM readout over to_dense_batch'd graphs.

Strategy (8 NeuronCores, SPMD single program):
 - Host: per-graph lengths from sorted `index`; graphs globally sorted by
   length (desc) and dealt round-robin to 8 cores, so all cores share one
   step schedule N_t = ceil(#active_global(t)/8). Host densifies x into a
   block-major padded tensor per core (bf16).
 - Device: per time-block, DMA-xbar-transpose loads x-dense as
   [128 = feat + 64*(t%2), cols]; per step, 4 accumulating bf16 matmul
   pairs compute gate preactivations for the active column prefix,
   ScalarE applies sigmoid/tanh (bias folded in), VectorE does the cell
   update, and each graph's final h is snapshotted into an output tile
   via a predicated copy at its true last step.
 - Host: gather per-core outputs, invert the deal/sort permutation.
"""

import numpy as np
import ml_dtypes

MAXLEN = 100
B = 8192
NCORES = 8
G = B // NCORES          # graph columns per core
H = 64
F = 64
TW = 20                  # steps per time block (even)
CHUNK = 512              # matmul free-dim chunk (psum bank)

_CACHE = {}


def _build_and_compile(schedule, weights):
    """Build the Bass program for a given (global) schedule. Returns nc."""
    import concourse.bacc as bacc
    import concourse.mybir as mybir
    from concourse import tile

    N_t, blocks, snap = schedule  # N_t: list; blocks: [(t0, nsteps, Wb, row0)]; snap: [(lo, hi, moff)]
    (wfi_x, wfi_h), (wog_x, wog_h), b_fi, b_og, sc_og = weights
    bf16 = mybir.dt.bfloat16
    f32 = mybir.dt.float32
    T_end = len(N_t)
    ROWS_TOT = sum(Wb * nst // 2 for (_, nst, Wb, _) in blocks)
    MW = sum(hi - lo for pieces in snap for (_, lo, hi, _) in pieces)
    XT_W = max(Wb * nst // 2 for (_, nst, Wb, _) in blocks)

    nc = bacc.Bacc("TRN2", target_bir_lowering=False)
    xd_d = nc.dram_tensor("xd", [128, ROWS_TOT], bf16, kind="ExternalInput")
    msk_d = nc.dram_tensor("msk", [64, max(MW, 1)], mybir.dt.uint8, kind="ExternalInput")
    out_d = nc.dram_tensor("outh", [64, G], bf16, kind="ExternalOutput")

    wfix_d = nc.dram_tensor("wfix", [128, 128], bf16, kind="ExternalInput")
    wogx_d = nc.dram_tensor("wogx", [128, 128], bf16, kind="ExternalInput")
    wfih_d = nc.dram_tensor("wfih", [64, 128], bf16, kind="ExternalInput")
    wogh_d = nc.dram_tensor("wogh", [64, 128], bf16, kind="ExternalInput")
    bfi_d = nc.dram_tensor("bfi", [128, 1], f32, kind="ExternalInput")
    bog_d = nc.dram_tensor("bog", [128, 1], f32, kind="ExternalInput")
    scog_d = nc.dram_tensor("scog", [128, 1], f32, kind="ExternalInput")

    Sig = mybir.ActivationFunctionType.Sigmoid
    Tanh = mybir.ActivationFunctionType.Tanh
    Mult = mybir.AluOpType.mult
    Add = mybir.AluOpType.add

    with tile.TileContext(nc) as tc:
        with tc.tile_pool(name="state", bufs=1) as sp, \
             tc.tile_pool(name="xblk", bufs=2) as xp, \
             tc.tile_pool(name="psum", bufs=2, space="PSUM") as pp:
            wfix = sp.tile([128, 128], bf16)
            nc.sync.dma_start(out=wfix, in_=wfix_d.ap())
            wogx = sp.tile([128, 128], bf16)
            nc.sync.dma_start(out=wogx, in_=wogx_d.ap())
            wfih = sp.tile([64, 128], bf16)
            nc.sync.dma_start(out=wfih, in_=wfih_d.ap())
            wogh = sp.tile([64, 128], bf16)
            nc.sync.dma_start(out=wogh, in_=wogh_d.ap())
            bfi = sp.tile([128, 1], f32)
            nc.sync.dma_start(out=bfi, in_=bfi_d.ap())
            bog = sp.tile([128, 1], f32)
            nc.sync.dma_start(out=bog, in_=bog_d.ap())
            scog = sp.tile([128, 1], f32)
            nc.sync.dma_start(out=scog, in_=scog_d.ap())
            mskt = sp.tile([64, max(MW, 1)], mybir.dt.uint8)
            nc.sync.dma_start(out=mskt, in_=msk_d.ap())

            h, cg, sfi, so, tc_t, fc, ig, outh = ({} for _ in range(8))
            for k in range(2):
                h[k] = sp.tile([64, CHUNK], bf16, tag=f"h{k}", name=f"h{k}")
                cg[k] = sp.tile([64, CHUNK], f32, tag=f"cg{k}", name=f"cg{k}")
                sfi[k] = sp.tile([128, CHUNK], f32, tag=f"sfi{k}", name=f"sfi{k}")
                so[k] = sp.tile([128, CHUNK], f32, tag=f"so{k}", name=f"so{k}")
                tc_t[k] = sp.tile([64, CHUNK], f32, tag=f"tc{k}", name=f"tc{k}")
                fc[k] = sp.tile([64, CHUNK], f32, tag=f"fc{k}", name=f"fc{k}")
                ig[k] = sp.tile([64, CHUNK], f32, tag=f"ig{k}", name=f"ig{k}")
                outh[k] = sp.tile([64, CHUNK], bf16, tag=f"oh{k}", name=f"oh{k}")
                nc.vector.memset(h[k][:, :], 0.0)
                nc.vector.memset(cg[k][:, :], 0.0)
                nc.vector.memset(outh[k][:, :], 0.0)

            for (t0, nsteps, Wb, row0) in blocks:
                rows_b = Wb * nsteps // 2
                xt = xp.tile([128, XT_W], bf16, tag="xt")
                nc.sync.dma_start(
                    out=xt[:, 0:rows_b], in_=xd_d.ap()[:, row0:row0 + rows_b])

                for ts in range(nsteps):
                    t = t0 + ts
                    n = N_t[t]
                    if n == 0:
                        continue
                    par = ts % 2
                    # work items: (psum_tag, state_tile, p0, p1); tail steps
                    # split the lone chunk into two pieces on separate psum
                    # banks so their ACT/DVE chains can interleave
                    if n > CHUNK:
                        work = [(0, 0, 0, CHUNK), (1, 1, 0, n - CHUNK)]
                    elif n >= 128:
                        m = (n // 2 + 1) & ~1
                        work = [(0, 0, 0, m), (1, 0, m, n)]
                    else:
                        work = [(0, 0, 0, n)]
                    fi_ps, og_ps = {}, {}
                    for (kt, km, p0, p1) in work:
                        w = p1 - p0
                        c0 = CHUNK * km + p0
                        fi_ps[kt] = pp.tile([128, CHUNK], f32, tag=f"fi{kt}", name=f"fi{kt}")
                        og_ps[kt] = pp.tile([128, CHUNK], f32, tag=f"og{kt}", name=f"og{kt}")
                        xs = xt[par * 64:(par + 1) * 64,
                                ts // 2 * Wb + c0:
                                ts // 2 * Wb + c0 + w]
                        nc.tensor.matmul(out=fi_ps[kt][:, 0:w],
                                         lhsT=wfix[par * 64:(par + 1) * 64, :],
                                         rhs=xs, start=True, stop=False)
                        nc.tensor.matmul(out=fi_ps[kt][:, 0:w],
                                         lhsT=wfih[:, :],
                                         rhs=h[km][:, p0:p1], start=False, stop=True)
                        nc.tensor.matmul(out=og_ps[kt][:, 0:w],
                                         lhsT=wogx[par * 64:(par + 1) * 64, :],
                                         rhs=xs, start=True, stop=False)
                        nc.tensor.matmul(out=og_ps[kt][:, 0:w],
                                         lhsT=wogh[:, :],
                                         rhs=h[km][:, p0:p1], start=False, stop=True)
                    for (kt, km, p0, p1) in work:
                        w = p1 - p0
                        nc.scalar.activation(out=sfi[km][:, p0:p1], in_=fi_ps[kt][:, 0:w],
                                             func=Sig, bias=bfi[:, :])
                        nc.scalar.activation(out=so[km][:, p0:p1], in_=og_ps[kt][:, 0:w],
                                             func=Sig, bias=bog[:, :], scale=scog[:, :])
                    for (kt, km, p0, p1) in work:
                        nc.vector.scalar_tensor_tensor(
                            out=fc[km][:, p0:p1], in0=cg[km][:, p0:p1], scalar=0.0,
                            in1=sfi[km][0:64, p0:p1], op0=Add, op1=Mult)
                        nc.vector.scalar_tensor_tensor(
                            out=ig[km][:, p0:p1], in0=so[km][64:128, p0:p1], scalar=-0.5,
                            in1=sfi[km][64:128, p0:p1], op0=Add, op1=Mult)
                        nc.vector.scalar_tensor_tensor(
                            out=cg[km][:, p0:p1], in0=ig[km][:, p0:p1], scalar=2.0,
                            in1=fc[km][:, p0:p1], op0=Mult, op1=Add)
                    for (kt, km, p0, p1) in work:
                        nc.scalar.activation(out=tc_t[km][:, p0:p1], in_=cg[km][:, p0:p1], func=Tanh)
                        nc.vector.tensor_tensor(out=h[km][:, p0:p1], in0=so[km][0:64, p0:p1],
                                                in1=tc_t[km][:, p0:p1], op=Mult)
                    for (kk, lo, hi, moff) in snap[t]:
                        nc.vector.copy_predicated(
                            out=outh[kk][:, lo:hi],
                            mask=mskt[:, moff:moff + (hi - lo)],
                            data=h[kk][:, lo:hi])

            nc.sync.dma_start(out=out_d.ap()[:, 0:CHUNK], in_=outh[0][:, :])
            nc.sync.dma_start(out=out_d.ap()[:, CHUNK:G], in_=outh[1][:, :])
    nc.compile()
    return nc


def _plan(lens):
    """Global schedule from capped lengths [B]. Returns (order, schedule helpers)."""
    order = np.argsort(-lens, kind="stable")
    lens_sorted = lens[order]
    T_end = int(lens_sorted.max())
    # per-core sorted lengths: core c, col j -> lens_sorted[8j + c]
    len_c = lens_sorted.reshape(G, NCORES).T  # [NCORES, G]
    # n_c(t) = #cols with len > t
    t_ax = np.arange(T_end + 1)
    n_c = (len_c[:, :, None] > t_ax[None, None, :]).sum(axis=1)  # [NCORES, T_end+1]
    N_t = n_c.max(axis=0)  # [T_end+1]; N_t[T_end] == 0
    # time blocks
    blocks = []
    row0 = 0
    t0 = 0
    while t0 < T_end:
        nsteps = min(TW, T_end - t0)
        if nsteps % 2:
            nsteps += 1  # keep even; schedule N_t beyond T_end is 0-pad
        Wb = int(np.ceil(N_t[t0] / 16) * 16)
        blocks.append((t0, nsteps, Wb, row0))
        row0 += Wb * nsteps // 2
        t0 += nsteps
    # snapshot ranges + masks
    snap = []
    moff = 0
    mask_cols = []
    for t in range(T_end):
        nt1 = n_c[:, t + 1] if t + 1 <= T_end else np.zeros(NCORES, np.int64)
        lo = int(nt1.min())
        hi = int(n_c[:, t].max())
        pieces = []
        if hi > lo:
            m = np.zeros((NCORES, hi - lo), np.uint8)
            for c in range(NCORES):
                a, b_ = int(nt1[c]), int(n_c[c, t])
                m[c, max(a - lo, 0):max(b_ - lo, 0)] = 1
            mask_cols.append(m)
            for k in range(2):
                plo = max(lo, 512 * k)
                phi = min(hi, 512 * (k + 1))
                if phi > plo:
                    pieces.append((k, plo - 512 * k, phi - 512 * k,
                                   moff + (plo - lo)))
            moff += hi - lo
        snap.append(pieces)
    masks = (np.concatenate(mask_cols, axis=1) if mask_cols
             else np.zeros((NCORES, 1), np.uint8))
    # pad schedule for block overhang (nsteps even rounding)
    N_pad = list(N_t[:T_end])
    total_steps = sum(ns for (_, ns, _, _) in blocks)
    while len(N_pad) < total_steps:
        N_pad.append(0)
        snap.append([])
    # drop zero-width steps from the tail of the schedule
    sched_N = [int(x) for x in N_pad]
    return order, len_c, n_c, sched_N, blocks, snap, masks


LAST_RUN = {}


def _install_ntff_shim():
    import sys, types
    if "antenv.axon_hooks" in sys.modules:
        return
    try:
        from trn_agent_boot.trn_boot import _ntff_profile_via_ctypes
        hook = _ntff_profile_via_ctypes("/opt/axon/libaxon_pjrt.so")
    except Exception:
        hook = None
    m = types.ModuleType("antenv.axon_hooks")
    m._hook = hook
    m.get_axon_ntff_profile_hook = lambda: m._hook
    m.set_axon_ntff_profile_hook = lambda h: setattr(m, "_hook", h)
    sys.modules["antenv.axon_hooks"] = m


def kernel(x, W_ih, W_hh, b_ih, b_hh, index, dim_size, _trace=False):
    from concourse.bass_utils import run_bass_kernel_spmd
    if _trace:
        import concourse.bass_utils as _bu
        _install_ntff_shim()
        _bu.upload_artifacts = lambda d: d  # no bucket in this container

    x = np.asarray(x)
    index = np.asarray(index).astype(np.int64)
    W_ih = np.asarray(W_ih, dtype=np.float32)
    W_hh = np.asarray(W_hh, dtype=np.float32)
    b_ih = np.asarray(b_ih, dtype=np.float32)
    b_hh = np.asarray(b_hh, dtype=np.float32)

    assert int(dim_size) == B, f"kernel hardcodes B={B}, got dim_size={int(dim_size)}"
    counts = np.bincount(index, minlength=B).astype(np.int64)
    offsets = np.concatenate([[0], np.cumsum(counts)[:-1]])
    lens = np.minimum(counts, MAXLEN)

    order, len_c, n_c, N_t, blocks, snap, masks = _plan(lens)

    # --- weights (torch gate order i,f,g,o -> ours f,i / o,g) ---
    b = (b_ih + b_hh).reshape(4, H)
    Wi, Wf, Wg, Wo = W_ih.reshape(4, H, F)
    Ui, Uf, Ug, Uo = W_hh.reshape(4, H, H)
    bf16 = ml_dtypes.bfloat16

    # ih stationaries duplicated at both parity halves (x-slices alternate
    # partition halves); hh stationaries at parts 0:64 (h lives there).
    wfi_x = np.concatenate([np.concatenate([Wf.T, Wi.T], 1)] * 2, 0).astype(bf16)
    wog_x = np.concatenate([np.concatenate([Wo.T, Wg.T], 1)] * 2, 0).astype(bf16)
    wfi_h = np.concatenate([Uf.T, Ui.T], 1).astype(bf16)  # [64, 128]
    wog_h = np.concatenate([Uo.T, Ug.T], 1).astype(bf16)
    b_fi = np.concatenate([b[1], b[0]]).reshape(128, 1).astype(np.float32)
    b_og = np.concatenate([b[3], 2.0 * b[2]]).reshape(128, 1).astype(np.float32)
    sc_og = np.concatenate([np.ones(64), 2.0 * np.ones(64)]).reshape(128, 1).astype(np.float32)

    # --- per-core dense input (block-major) ---
    x_bf = x.astype(bf16)
    T_end = len(N_t)
    in_maps = []
    for c in range(NCORES):
        gids = order[np.arange(G) * NCORES + c]     # col j -> graph id
        lens_cj = len_c[c]                          # [G]
        offs_cj = offsets[gids]
        parts = []
        for (t0, nsteps, Wb, row0) in blocks:
            tsl = np.arange(t0, t0 + nsteps)
            node = offs_cj[:Wb, None] + tsl[None, :]             # [Wb, nsteps]
            valid = tsl[None, :] < lens_cj[:Wb, None]
            node = np.clip(node, 0, x.shape[0] - 1)
            blk = np.where(valid[:, :, None], x_bf[node], bf16(0))  # [Wb, nsteps, 64]
            # time-major rows: row r = taupair*Wb + g  -> per-step rhs contiguous
            blk = blk.reshape(Wb, nsteps // 2, 128).transpose(1, 0, 2)
            parts.append(blk.reshape(nsteps // 2 * Wb, 128))
        xd = np.ascontiguousarray(np.concatenate(parts, axis=0).T)
        msk = np.ascontiguousarray(
            np.broadcast_to(masks[c][None, :], (64, masks.shape[1])))
        in_maps.append({"xd": xd, "msk": msk,
                        "wfix": wfi_x, "wogx": wog_x, "wfih": wfi_h,
                        "wogh": wog_h, "bfi": b_fi, "bog": b_og, "scog": sc_og})

    key = (tuple(N_t), tuple(blocks), repr(snap),
           W_ih.tobytes(), W_hh.tobytes(), b_ih.tobytes(), b_hh.tobytes())
    import hashlib
    key = hashlib.sha1(repr(key[:3]).encode() + key[3] + key[4] + key[5] + key[6]).hexdigest()
    if key not in _CACHE:
        _CACHE[key] = _build_and_compile(
            (N_t, blocks, snap),
            ((wfi_x, wfi_h), (wog_x, wog_h), b_fi, b_og, sc_og))
    nc = _CACHE[key]

    res = run_bass_kernel_spmd(nc, in_maps, core_ids=list(range(NCORES)),
                               trace=_trace)
    LAST_RUN["res"] = res

    out = np.zeros((B, H), np.float32)
    for c in range(NCORES):
        hT = res.results[c]["outh"].astype(np.float32)  # [64, G]
        gids = order[np.arange(G) * NCORES + c]
        out[gids] = hT.T
    return out



# revision 12
# speedup vs baseline: 1.0933x; 1.0933x over previous
"""Trainium2 Bass kernel: masked-LSTM readout over to_dense_batch'd graphs.

v2 design (per core, SPMD over 8 cores):
 - Host: graphs sorted by length desc, dealt round-robin to 8 cores so all
   cores share one step schedule N_t. Columns within a core are length-sorted
   (prefix-active). x densified to fp16 step-major slabs [64, Wb] per step.
 - Device, per step t, per column-piece p (prefix staircase):
     * one fused 128-contraction matmul per gate-pair: rhs = xh tile
       [x(t) at parts 0:64 | h(t-1) at parts 64:128], stationary packs
       [W_x ; U_h] with the tanh-gate rows prescaled by 2.
     * sigma_ig = sigmoid(psum_ig + b_ig), sigma_fo likewise (ACT, bias AP,
       fp16 out).  s := sigmoid(2*pre_g)  => tanh(pre_g) = 2s - 1.
     * DVE (all fp16, 2x/4x modes): S=(s-.5)*4 ; Bv=S*i ; A=f*C ; C'=A+Bv
       with C := 2c, so C' = 2c'.
     * ACT: T = tanh(0.5 * C') = tanh(c') directly (same act table).
     * DVE: h = T * o written into the next step's xh window (parts 64:128).
     * Dying columns' h snapshotted into outh via copy_predicated.
"""

import numpy as np

MAXLEN = 100
B = 8192
NCORES = 8
G = B // NCORES          # graph columns per core
H = 64
F = 64
TW = 10                  # steps per time block
PIECE_BOUNDS = [0, 512, 1024]   # prefix column-piece boundaries

_CACHE = {}


def _build_and_compile(schedule, weights):
    import concourse.bacc as bacc
    import concourse.mybir as mybir
    from concourse import tile

    N_t, blocks, snap = schedule
    # blocks: [(t0, nsteps, Wb, row0)]
    fp16 = mybir.dt.float16
    f32 = mybir.dt.float32
    T_end = len(N_t)
    ROWS_TOT = sum(Wb * nsteps for (_, nsteps, Wb, _) in blocks)
    MW_TOT = sum(hi - lo for t in range(len(snap)) for (_, lo, hi, _) in snap[t])
    XW = max(Wb * nsteps for (_, nsteps, Wb, _) in blocks)

    npieces = len(PIECE_BOUNDS) - 1
    psum_bufs = max(1, min(2, 8 // (2 * npieces)))

    nc = bacc.Bacc("TRN2", target_bir_lowering=False)
    xd_d = nc.dram_tensor("xd", [64, ROWS_TOT], fp16, kind="ExternalInput")
    msk_d = nc.dram_tensor("msk", [64, max(MW_TOT, 1)], mybir.dt.uint8, kind="ExternalInput")
    out_d = nc.dram_tensor("outh", [64, G], fp16, kind="ExternalOutput")
    wig_d = nc.dram_tensor("wig", [128, 128], fp16, kind="ExternalInput")
    wfo_d = nc.dram_tensor("wfo", [128, 128], fp16, kind="ExternalInput")
    big_d = nc.dram_tensor("big", [128, 1], f32, kind="ExternalInput")
    bfo_d = nc.dram_tensor("bfo", [128, 1], f32, kind="ExternalInput")

    Sig = mybir.ActivationFunctionType.Sigmoid
    Tanh = mybir.ActivationFunctionType.Tanh
    Mult = mybir.AluOpType.mult
    Add = mybir.AluOpType.add
    Sub = mybir.AluOpType.subtract

    with tile.TileContext(nc) as tc:
        with tc.tile_pool(name="state", bufs=1) as sp, \
             tc.tile_pool(name="gates", bufs=2) as gp, \
             tc.tile_pool(name="psum", bufs=psum_bufs, space="PSUM") as pp:
            wig = sp.tile([128, 128], fp16)
            nc.sync.dma_start(out=wig, in_=wig_d.ap())
            wfo = sp.tile([128, 128], fp16)
            nc.sync.dma_start(out=wfo, in_=wfo_d.ap())
            big = sp.tile([128, 1], f32)
            nc.sync.dma_start(out=big, in_=big_d.ap())
            bfo = sp.tile([128, 1], f32)
            nc.sync.dma_start(out=bfo, in_=bfo_d.ap())
            mskt_f = sp.tile([128, max(MW_TOT, 1)], mybir.dt.uint8, name="mskt")
            mskt = mskt_f[64:128, :]
            nc.sync.dma_start(out=mskt, in_=msk_d.ap())

            # persistent state (C at base partition 0, pairs with f in tt1)
            C = sp.tile([64, G], fp16, name="C")       # C = 2c
            outh_f = sp.tile([128, G], fp16, name="outh")
            outh = outh_f[64:128, :]
            nc.vector.memset(C[:, :], 0.0)
            nc.vector.memset(outh, 0.0)

            # xh staging: manual double buffer, blocks alternate
            xh = [sp.tile([128, XW], fp16, name=f"xh{k}") for k in range(2)]
            # zero h-half of block0 window0 (h(-1) = 0)
            nc.vector.memset(xh[0][64:128, 0:blocks[0][2]], 0.0)

            for bi_, (t0, nsteps, Wb, row0) in enumerate(blocks):
                xt = xh[bi_ % 2]
                rows_b = Wb * nsteps
                nc.sync.dma_start(out=xt[0:64, 0:rows_b],
                                  in_=xd_d.ap()[:, row0:row0 + rows_b])

                for ts in range(nsteps):
                    t = t0 + ts
                    if t >= T_end or N_t[t] == 0:
                        continue
                    n = N_t[t]
                    # next-step window (h target): maybe in next block
                    if ts + 1 < nsteps:
                        nxt, nwin = xt, (ts + 1) * Wb
                    else:
                        nxt, nwin = xh[(bi_ + 1) % 2], 0
                    win = ts * Wb

                    pw = []          # (p, p0, w) active pieces this step
                    for p in range(npieces):
                        p0, p1 = PIECE_BOUNDS[p], PIECE_BOUNDS[p + 1]
                        w = min(n, p1) - p0
                        if w > 0:
                            pw.append((p, p0, w))

                    ps_ig, ps_fo = {}, {}
                    for (p, p0, w) in pw:
                        ps_ig[p] = pp.tile([128, 512], f32, tag=f"ig{p}", name=f"ig{p}")
                        ps_fo[p] = pp.tile([128, 512], f32, tag=f"fo{p}", name=f"fo{p}")
                        rhs = xt[:, win + p0: win + p0 + w]
                        nc.tensor.matmul(out=ps_fo[p][:, 0:w], lhsT=wfo[:, :],
                                         rhs=rhs, start=True, stop=True)
                        nc.tensor.matmul(out=ps_ig[p][:, 0:w], lhsT=wig[:, :],
                                         rhs=rhs, start=True, stop=True)
                    sfo, sig_, Tt = {}, {}, {}
                    for (p, p0, w) in pw:
                        sfo[p] = gp.tile([128, 512], fp16, tag=f"sfo{p}", name=f"sfo{p}")
                        nc.scalar.activation(out=sfo[p][:, 0:w], in_=ps_fo[p][:, 0:w],
                                             func=Sig, bias=bfo[:, :])
                        sig_[p] = gp.tile([128, 512], fp16, tag=f"sig{p}", name=f"sig{p}")
                        nc.scalar.activation(out=sig_[p][:, 0:w], in_=ps_ig[p][:, 0:w],
                                             func=Sig, bias=big[:, :])
                    for (p, p0, w) in pw:
                        # A = f * C   (f at sfo[0:64], C base 0) -> A at base 64
                        A = gp.tile([128, 512], fp16, tag=f"A{p}", name=f"A{p}")
                        nc.vector.tensor_tensor(out=A[64:128, 0:w],
                                                in0=sfo[p][0:64, 0:w],
                                                in1=C[:, p0:p0 + w], op=Mult)
                        # S = (s - 0.5) * 4   (s at sig_[0:64]) -> S at base 64
                        S = gp.tile([128, 512], fp16, tag=f"S{p}", name=f"S{p}")
                        nc.vector.tensor_scalar(out=S[64:128, 0:w],
                                                in0=sig_[p][0:64, 0:w],
                                                scalar1=0.5, scalar2=4.0,
                                                op0=Sub, op1=Mult)
                        # Bv = S * i  (i at sig_[64:128], S at 64) -> base 64
                        Bv = gp.tile([128, 512], fp16, tag=f"B{p}", name=f"B{p}")
                        nc.vector.tensor_tensor(out=Bv[64:128, 0:w],
                                                in0=S[64:128, 0:w],
                                                in1=sig_[p][64:128, 0:w], op=Mult)
                        # C' = A + Bv  (both base 64; out base 0 into C)
                        nc.vector.tensor_tensor(out=C[:, p0:p0 + w],
                                                in0=A[64:128, 0:w],
                                                in1=Bv[64:128, 0:w], op=Add)
                    for (p, p0, w) in pw:
                        Tt[p] = gp.tile([128, 512], fp16, tag=f"T{p}", name=f"T{p}")
                        nc.scalar.activation(out=Tt[p][64:128, 0:w],
                                             in_=C[:, p0:p0 + w],
                                             func=Tanh, scale=0.5)
                    for (p, p0, w) in pw:
                        # h = T * o  (o at sfo[64:128], T at 64) -> next window
                        nc.vector.tensor_tensor(
                            out=nxt[64:128, nwin + p0: nwin + p0 + w],
                            in0=Tt[p][64:128, 0:w], in1=sfo[p][64:128, 0:w], op=Mult)
                    for (kk, lo, hi, moff) in snap[t]:
                        nc.vector.copy_predicated(
                            out=outh[:, lo:hi],
                            mask=mskt[:, moff:moff + (hi - lo)],
                            data=nxt[64:128, nwin + lo: nwin + hi])

            nc.sync.dma_start(out=out_d.ap()[:, :], in_=outh[:, :])
    nc.compile()
    return nc


def _plan(lens):
    """Global schedule from capped lengths [B]."""
    order = np.argsort(-lens, kind="stable")
    lens_sorted = lens[order]
    T_end = int(lens_sorted.max())
    len_c = lens_sorted.reshape(G, NCORES).T  # [NCORES, G]
    t_ax = np.arange(T_end + 1)
    n_c = (len_c[:, :, None] > t_ax[None, None, :]).sum(axis=1)  # [NCORES, T+1]
    N_t = n_c.max(axis=0)
    # time blocks; Wb covers the P5 write of the previous step's width
    blocks = []
    row0 = 0
    t0 = 0
    while t0 < T_end:
        nsteps = min(TW, T_end - t0)
        Wb = int(np.ceil(N_t[max(t0 - 1, 0)] / 16) * 16)
        blocks.append((t0, nsteps, Wb, row0))
        row0 += Wb * nsteps
        t0 += nsteps
    # trailing pad block: one window for the final P5 write
    Wb_pad = int(np.ceil(N_t[T_end - 1] / 16) * 16)
    blocks.append((T_end, 1, Wb_pad, row0))
    row0 += Wb_pad

    # snapshot ranges + masks (mask marks cols whose len == t+1)
    snap = []
    moff = 0
    mask_cols = []
    for t in range(T_end):
        nt1 = n_c[:, t + 1] if t + 1 <= T_end else np.zeros(NCORES, np.int64)
        lo = int(nt1.min())
        hi = int(n_c[:, t].max())
        pieces = []
        if hi > lo:
            m = np.zeros((NCORES, hi - lo), np.uint8)
            for c in range(NCORES):
                a, b_ = int(nt1[c]), int(n_c[c, t])
                m[c, max(a - lo, 0):max(b_ - lo, 0)] = 1
            mask_cols.append(m)
            pieces.append((0, lo, hi, moff + 0))
            moff += hi - lo
        snap.append(pieces)
    masks = (np.concatenate(mask_cols, axis=1) if mask_cols
             else np.zeros((NCORES, 1), np.uint8))
    N_list = [int(x) for x in N_t[:T_end]]
    return order, len_c, n_c, N_list, blocks, snap, masks


LAST_RUN = {}


def _install_ntff_shim():
    import sys, types
    if "antenv.axon_hooks" in sys.modules:
        return
    try:
        from trn_agent_boot.trn_boot import _ntff_profile_via_ctypes
        hook = _ntff_profile_via_ctypes("/opt/axon/libaxon_pjrt.so")
    except Exception:
        hook = None
    m = types.ModuleType("antenv.axon_hooks")
    m._hook = hook
    m.get_axon_ntff_profile_hook = lambda: m._hook
    m.set_axon_ntff_profile_hook = lambda h: setattr(m, "_hook", h)
    sys.modules["antenv.axon_hooks"] = m


def kernel(x, W_ih, W_hh, b_ih, b_hh, index, dim_size, _trace=False):
    from concourse.bass_utils import run_bass_kernel_spmd
    if _trace:
        import concourse.bass_utils as _bu
        _install_ntff_shim()
        _bu.upload_artifacts = lambda d: d

    x = np.asarray(x)
    index = np.asarray(index).astype(np.int64)
    W_ih = np.asarray(W_ih, dtype=np.float32)
    W_hh = np.asarray(W_hh, dtype=np.float32)
    b_ih = np.asarray(b_ih, dtype=np.float32)
    b_hh = np.asarray(b_hh, dtype=np.float32)

    assert int(dim_size) == B, f"kernel hardcodes B={B}, got dim_size={int(dim_size)}"
    counts = np.bincount(index, minlength=B).astype(np.int64)
    offsets = np.concatenate([[0], np.cumsum(counts)[:-1]])
    lens = np.minimum(counts, MAXLEN)

    order, len_c, n_c, N_t, blocks, snap, masks = _plan(lens)

    # --- weights (torch gate order i,f,g,o) ---
    b = (b_ih + b_hh).reshape(4, H)
    Wi, Wf, Wg, Wo = W_ih.reshape(4, H, F)
    Ui, Uf, Ug, Uo = W_hh.reshape(4, H, H)
    fp16 = np.float16

    # stationary [K=128 (x 0:64, h 64:128), M=128]; ig tile = [g(x2) | i],
    # fo tile = [f | o] so tensor_tensor operand bases line up on device.
    w_ig = np.zeros((128, 128), np.float32)
    w_ig[0:64, 0:64] = 2.0 * Wg.T
    w_ig[64:128, 0:64] = 2.0 * Ug.T
    w_ig[0:64, 64:128] = Wi.T
    w_ig[64:128, 64:128] = Ui.T
    w_fo = np.zeros((128, 128), np.float32)
    w_fo[0:64, 0:64] = Wf.T
    w_fo[64:128, 0:64] = Uf.T
    w_fo[0:64, 64:128] = Wo.T
    w_fo[64:128, 64:128] = Uo.T
    w_ig = w_ig.astype(fp16)
    w_fo = w_fo.astype(fp16)
    b_ig = np.concatenate([2.0 * b[2], b[0]]).reshape(128, 1).astype(np.float32)
    b_fo = np.concatenate([b[1], b[3]]).reshape(128, 1).astype(np.float32)

    # --- per-core dense x slabs (step-major rows) ---
    x16 = x.astype(fp16)
    in_maps = []
    for c in range(NCORES):
        gids = order[np.arange(G) * NCORES + c]
        lens_cj = len_c[c]
        offs_cj = offsets[gids]
        parts = []
        for (t0, nsteps, Wb, row0) in blocks:
            tsl = np.arange(t0, t0 + nsteps)
            node = offs_cj[:Wb, None] + tsl[None, :]             # [Wb, nsteps]
            valid = tsl[None, :] < lens_cj[:Wb, None]
            node = np.clip(node, 0, x.shape[0] - 1)
            blk = np.where(valid[:, :, None], x16[node], fp16(0))  # [Wb,ns,64]
            parts.append(blk.transpose(1, 0, 2).reshape(nsteps * Wb, 64))
        xd = np.ascontiguousarray(np.concatenate(parts, axis=0).T)  # [64, ROWS]
        msk = np.ascontiguousarray(
            np.broadcast_to(masks[c][None, :], (64, masks.shape[1])))
        in_maps.append({"xd": xd, "msk": msk, "wig": w_ig, "wfo": w_fo,
                        "big": b_ig, "bfo": b_fo})

    import hashlib
    key = hashlib.sha1(
        (repr((N_t, blocks, repr(snap), PIECE_BOUNDS, TW)).encode()
         + w_ig.tobytes() + w_fo.tobytes() + b_ig.tobytes() + b_fo.tobytes())
    ).hexdigest()
    if key not in _CACHE:
        _CACHE[key] = _build_and_compile((N_t, blocks, snap),
                                         (w_ig, w_fo, b_ig, b_fo))
    nc = _CACHE[key]

    res = run_bass_kernel_spmd(nc, in_maps, core_ids=list(range(NCORES)),
                               trace=_trace)
    LAST_RUN["res"] = res

    out = np.zeros((B, H), np.float32)
    for c in range(NCORES):
        hT = res.results[c]["outh"].astype(np.float32)  # [64, G]
        gids = order[np.arange(G) * NCORES + c]
        out[gids] = hT.T
    return out
